# revision 31
# baseline (speedup 1.0000x reference)
"""Trainium2 Bass kernel for Conv2d_XnorPP_SCA (binarized 3x3 conv).

Computes: out = conv2d(sign(x), round(tanh(w)), stride=1, pad=1) * alpha
  x: [64, 64, 112, 112] f32, w: [64, 64, 3, 3] f32, alpha: [64,1,1] f32

Strategy (per NeuronCore, data-parallel over batch, 8 images/core):
  - Zero-padded flat layout: each image is sign-binarized (fp8e4, exact
    for +-1) into a [64, 114*114] SBUF tile with zero borders; every 3x3
    tap then becomes a constant column offset, so the conv is 9
    PSUM-accumulated fp8 matmuls (K=Cin=64, M=Cout=64) per output tile.
  - Two images are resident at once (partitions 0-63 / 64-127); stream
    (img, rowhalf rh) runs on PE quadrant (row=img, col=img^rh) so 4 matmul
    streams run concurrently and each PSUM bank is fully used (img0+img1
    halves of the same 4 output rows share a bank).
  - Weights are ternarized to {-1,0,1} (exact in fp8e4); alpha is applied
    per out-channel during PSUM evacuation (ScalarE activation scale /
    VectorE tensor_scalar_mul), so arbitrary alpha stays f32-exact.
  - PSUM is evacuated into [128, 56*112] staging laid out so partition =
    64*img + cout; the store DMA then writes 25KB fully contiguous per
    partition (vs 1792B interleaved runs), which is what HBM write
    bandwidth needs. Stores are dispatched via GpSimd SWDGE (idle engine,
    separate ring) so their evac-waits and descriptors never block the
    input loads on the sync-engine HWDGE ring; loads for pair k+1 are
    dispatched at pair k's start and self-pace via a 12-buffer chunk pool,
    with sign ops spread between evacuations on ScalarE.
"""

import numpy as np
import ml_dtypes

H = W = 112
WP = 114
P_COLS = WP * WP + 2  # 12998: +1 margin at each end
CIN = COUT = 64
N_CORES = 8
NI = 8  # images per core
ROWS_PER_CHUNK = 14  # input load/sign granularity
ROWS_PER_GROUP = 8  # output rows per PE group (2 psum halves x 4 rows)
GROUPS_PER_SG = 7  # groups per output staging supergroup (56 rows)


def build_nc(ni=NI):
    import concourse.bacc as bacc
    import concourse.mybir as mybir
    from concourse.tile import TileContext

    f32 = mybir.dt.float32
    fp8 = mybir.dt.float8e4

    nc = bacc.Bacc("TRN2", target_bir_lowering=False, debug=False)
    x_d = nc.dram_tensor("x", [ni, CIN, H, W], f32, kind="ExternalInput")
    w_d = nc.dram_tensor("w", [128, 9 * COUT], fp8, kind="ExternalInput")
    al_d = nc.dram_tensor("al", [128, 1], f32, kind="ExternalInput")
    o_d = nc.dram_tensor("out", [ni, COUT, H, W], f32, kind="ExternalOutput")

    x_flat = x_d.ap().rearrange("n c h w -> (n c) (h w)")
    npairs = ni // 2
    n_chunks = H // ROWS_PER_CHUNK  # 4
    n_groups = H // ROWS_PER_GROUP  # 14
    n_sg = n_groups // GROUPS_PER_SG  # 2

    with TileContext(nc) as tc:
        with (
            tc.tile_pool(name="wp", bufs=1) as wp,
            tc.tile_pool(name="inp", bufs=12) as inp,
            tc.tile_pool(name="pp", bufs=1) as pp,
            tc.tile_pool(name="op", bufs=2) as op,
            tc.tile_pool(name="psp", bufs=8, space="PSUM") as psp,
        ):
            w_sb = wp.tile([128, 9 * COUT], fp8, name="w_sb")
            nc.sync.dma_start(out=w_sb[:, :], in_=w_d.ap())
            al_sb = wp.tile([128, 1], f32, name="al_sb")
            nc.sync.dma_start(out=al_sb[:, :], in_=al_d.ap())

            p_tiles = []
            for i in range(2):
                pt = pp.tile([128, P_COLS], fp8, tag=f"p{i}", name=f"p{i}")
                # zero only the padding (sign overwrites the interior every
                # pair; borders stay zero for the kernel lifetime)
                nc.vector.memset(pt[:, 0:116], 0.0)
                gaps = pt[:, 116:116 + H * WP]
                gaps = gaps.rearrange("q (r w) -> q r w", w=WP)[:, :, W:WP]
                nc.vector.memset(gaps, 0.0)
                nc.vector.memset(pt[:, 116 + H * WP - 2:P_COLS], 0.0)
                p_tiles.append(pt)

            chunk_tiles = {}

            def emit_loads(pair):
                """Dispatch all x loads for `pair` (sync-engine HWDGE ring).

                Buffer WAR (12 bufs) self-paces them a full pair ahead."""
                for ci in range(n_chunks):
                    y0 = ci * ROWS_PER_CHUNK
                    st = inp.tile([128, ROWS_PER_CHUNK * W], f32, tag="xin",
                                  name="xin")
                    nc.sync.dma_start(
                        out=st[:, :],
                        in_=x_flat[pair * 128:(pair + 1) * 128,
                                   y0 * W:(y0 + ROWS_PER_CHUNK) * W],
                    )
                    chunk_tiles[(pair, ci)] = st

            def emit_signs(pair, cis):
                """Binarize chunks `cis` of `pair` into its p tile (ScalarE)."""
                p = p_tiles[pair % 2]
                for ci in cis:
                    y0 = ci * ROWS_PER_CHUNK
                    st = chunk_tiles.pop((pair, ci))
                    # sign(x) -> fp8e4, written at stride-114 interior positions
                    dst = p[:, 116 + y0 * WP: 116 + y0 * WP + ROWS_PER_CHUNK * WP]
                    dst = dst.rearrange("q (r w) -> q r w", w=WP)[:, :, :W]
                    src = st[:, :].rearrange("q (r w) -> q r w", w=W)
                    nc.scalar.activation(
                        out=dst, in_=src,
                        func=mybir.ActivationFunctionType.Sign)

            emit_loads(0)
            emit_signs(0, range(n_chunks))
            for pair in range(npairs):
                p = p_tiles[pair % 2]
                # ---- conv: groups of 8 output rows ----
                for sg in range(n_sg):
                    sg_rows = GROUPS_PER_SG * ROWS_PER_GROUP  # 56
                    # staging: partition = 64*img + cout, cols = (row, w);
                    # each partition's 56 rows are contiguous in DRAM.
                    st = op.tile([128, sg_rows * W], f32, tag="so", name="so")
                    for g7 in range(GROUPS_PER_SG):
                        g = sg * GROUPS_PER_SG + g7
                        r0 = 1 + g * ROWS_PER_GROUP  # first padded row of group
                        # 2 psum tiles, one per rowhalf; partitions 64*(img^rh)
                        q_tiles = []
                        for qi in range(2):
                            qt = psp.tile([128, 456], f32, tag="ps", name=f"ps{qi}",
                                          padded_shape=[128, 512])
                            q_tiles.append(qt)
                        for t in range(9):
                            ky, kx = divmod(t, 3)
                            lhs = [w_sb[0:64, t * 64:(t + 1) * 64],
                                   w_sb[64:128, t * 64:(t + 1) * 64]]
                            first, last = (t == 0), (t == 8)
                            for rh in range(2):  # row half: rows r0+4*rh..+3
                                s = (r0 + 4 * rh + ky - 1) * WP + kx
                                for img in range(2):
                                    col = img ^ rh
                                    qt = q_tiles[rh]
                                    out_ap = qt[64 * col:64 * col + 64, 0:456]
                                    rhs = p[64 * img:64 * (img + 1), s:s + 456]
                                    nc.tensor.matmul(
                                        out_ap, lhs[img], rhs,
                                        start=first, stop=last)
                        # evacuate into img-major staging
                        base = g7 * ROWS_PER_GROUP * W
                        # rh0 bank: partitions already = 64*img + c
                        src0 = q_tiles[0][:, 0:456]
                        src0 = src0.rearrange("q (r w) -> q r w", w=WP)[:, :, 1:1 + W]
                        dst0 = st[:, base:base + 4 * W]
                        dst0 = dst0.rearrange("q (r w) -> q r w", w=W)
                        nc.vector.tensor_scalar_mul(dst0, src0, al_sb[:, 0:1])
                        # rh1 bank: img1 in partitions 0-63, img0 in 64-127
                        src1 = q_tiles[1][:, 0:456]
                        src1 = src1.rearrange("q (r w) -> q r w", w=WP)[:, :, 1:1 + W]
                        dst1 = st[:, base + 4 * W:base + 8 * W]
                        dst1 = dst1.rearrange("q (r w) -> q r w", w=W)
                        nc.scalar.activation(
                            out=dst1[64:128], in_=src1[0:64],
                            func=mybir.ActivationFunctionType.Copy,
                            scale=al_sb[0:64, 0:1])
                        nc.vector.tensor_scalar_mul(
                            dst1[0:64], src1[64:128], al_sb[64:128, 0:1])
                        # split store: groups go out as soon as evacuated.
                        # Final band of the whole kernel is split finer so the
                        # unoverlapped drain tail is short.
                        last_band = (pair == npairs - 1 and sg == n_sg - 1)
                        cuts = (1, 3, 5, GROUPS_PER_SG - 1) if last_band \
                            else (2, GROUPS_PER_SG - 1)
                        if g7 in cuts:
                            idx = cuts.index(g7)
                            ga = 0 if idx == 0 else cuts[idx - 1] + 1
                            gb = g7 + 1
                            y0 = sg * sg_rows + ga * ROWS_PER_GROUP
                            rows = (gb - ga) * ROWS_PER_GROUP
                            dst4 = o_d.ap()[2 * pair:2 * pair + 2, :,
                                            y0:y0 + rows, :]
                            dst4 = dst4.rearrange("n c h w -> (n c) (h w)")
                            # SWDGE dispatch: GpSimd is otherwise idle, so
                            # the wait-for-evac before each store blocks no
                            # compute engine (scalar/sync dispatch would)
                            nc.gpsimd.dma_start(
                                out=dst4,
                                in_=st[:, ga * ROWS_PER_GROUP * W:
                                       gb * ROWS_PER_GROUP * W])
                        # next pair's input: loads dispatched at this pair's
                        # start; signs spread 2-at-a-time between evacs so
                        # neither signs nor PSUM recycling ever burst-block
                        # the scalar engine (keeps TensorE gapless)
                        if sg == 0 and pair + 1 < npairs:
                            if g7 == 0:
                                emit_loads(pair + 1)
                            if g7 in (2, 3, 4, 5):
                                emit_signs(pair + 1,
                                           [(g7 - 2) * 2, (g7 - 2) * 2 + 1])
    nc.compile()
    return nc


def pack_weights(weight, alpha):
    """Ternarize (round(tanh(w))), pack as [128, 9*64] fp8e4 lhsT.

    The ternary {-1, 0, 1} values are exact in fp8e4; alpha is NOT folded
    here -- it is applied per out-channel during PSUM evacuation."""
    wt = _ternarize(np.asarray(weight, dtype=np.float32))
    # lhsT[k=cin, t*64+cout]
    arr = wt.transpose(1, 2, 3, 0).reshape(CIN, 9 * COUT)
    pack = np.empty((128, 9 * COUT), dtype=ml_dtypes.float8_e4m3)
    pack[0:64] = arr.astype(ml_dtypes.float8_e4m3)
    pack[64:128] = pack[0:64]
    return pack


def _ternarize(w):
    try:
        import jax
        cpu = jax.devices("cpu")[0]
        with jax.default_device(cpu):
            import jax.numpy as jnp
            return np.asarray(jnp.round(jnp.tanh(jnp.asarray(w))))
    except Exception:
        return np.round(np.tanh(w.astype(np.float32))).astype(np.float32)


_NC_CACHE = {}


def _get_nc():
    if "nc" not in _NC_CACHE:
        _NC_CACHE["nc"] = build_nc(NI)
    return _NC_CACHE["nc"]


def _make_runner():
    """Build (once) a jitted shard_map callable running the NEFF on 8 cores.

    Mirrors concourse.bass2jax.run_bass_via_pjrt's multi-core path, but
    caches the jitted function so repeated calls skip retracing and inputs
    can be passed as device-resident jax arrays for timing.
    """
    if "runner" in _NC_CACHE:
        return _NC_CACHE["runner"]
    import jax
    import concourse.mybir as mybir
    from concourse import bass2jax
    from jax.sharding import Mesh, PartitionSpec
    from jax.experimental.shard_map import shard_map

    nc = _get_nc()
    bass2jax.install_neuronx_cc_hook()

    partition_name = (nc.partition_id_tensor.name
                      if nc.partition_id_tensor else None)
    in_names, out_names, out_avals, zero_shapes = [], [], [], []
    for alloc in nc.m.functions[0].allocations:
        if not isinstance(alloc, mybir.MemoryLocationSet):
            continue
        name = alloc.memorylocations[0].name
        if alloc.kind == "ExternalInput":
            if name != partition_name:
                in_names.append(name)
        elif alloc.kind == "ExternalOutput":
            out_names.append(name)
            shape = tuple(alloc.tensor_shape)
            dtype = mybir.dt.np(alloc.dtype)
            out_avals.append(jax.core.ShapedArray(shape, dtype))
            zero_shapes.append((shape, dtype))
    n_params = len(in_names)
    all_in_names = in_names + out_names
    if partition_name is not None:
        all_in_names = all_in_names + [partition_name]

    def _body(*args):
        operands = list(args)
        if partition_name is not None:
            operands.append(bass2jax.partition_id_tensor())
        outs = bass2jax._bass_exec_p.bind(
            *operands,
            out_avals=tuple(out_avals),
            in_names=tuple(all_in_names),
            out_names=tuple(out_names),
            lowering_input_output_aliases=(),
            sim_require_finite=True,
            sim_require_nnan=True,
            nc=nc,
        )
        return tuple(outs)

    devices = jax.devices()[:N_CORES]
    mesh = Mesh(np.asarray(devices), ("core",))
    n_outs = len(out_names)
    donate = tuple(range(n_params, n_params + n_outs))
    in_specs = (PartitionSpec("core"),) * (n_params + n_outs)
    out_specs = (PartitionSpec("core"),) * n_outs
    sharded = jax.jit(
        shard_map(_body, mesh=mesh, in_specs=in_specs, out_specs=out_specs,
                  check_rep=False),
        donate_argnums=donate, keep_unused=True)
    runner = {
        "fn": sharded, "mesh": mesh, "in_names": in_names,
        "out_names": out_names, "zero_shapes": zero_shapes,
        "n_params": n_params,
    }
    _NC_CACHE["runner"] = runner
    return runner


def make_concat_inputs(x, w_pack, alpha=None):
    """Per-core inputs concatenated on axis 0 (shard_map layout)."""
    xs = np.ascontiguousarray(x.reshape(N_CORES * NI, CIN, H, W))
    ws = np.concatenate([w_pack] * N_CORES, axis=0)
    if alpha is None:
        alpha = np.ones((COUT,), np.float32)
    al = np.tile(np.asarray(alpha, np.float32).reshape(COUT, 1), (2, 1))
    als = np.concatenate([al] * N_CORES, axis=0)
    return {"x": xs, "w": ws, "al": als}


def make_zeros():
    r = _make_runner()
    return [np.zeros((N_CORES * s[0], *s[1:]), d) for s, d in r["zero_shapes"]]


def run_concat(concat_by_name, zeros=None):
    """Run on 8 cores. Inputs may be numpy or device-resident jax arrays."""
    r = _make_runner()
    if zeros is None:
        zeros = make_zeros()
    args = [concat_by_name[n] for n in r["in_names"]] + list(zeros)
    out_arrs = r["fn"](*args)
    return out_arrs


def kernel(x, weight, alpha):
    x = np.asarray(x, dtype=np.float32)
    w_pack = pack_weights(weight, alpha)
    concat = make_concat_inputs(x, w_pack, alpha)
    out_arrs = run_concat(concat)
    out = np.asarray(out_arrs[0]).reshape(64, COUT, H, W)
    return out.astype(np.float32, copy=False)

